# revision 57
# baseline (speedup 1.0000x reference)
"""Trainium2 Bass kernel for MultiHeadAttention (B=2, N=2048, DIM=1024, H=16).

Sharding: 8 cores = 2 batches x 4 head-groups (4 heads each). Each core
computes qkv projections for its head slice, attention, and a partial
output projection; the host sums the 4 partials per batch, applies the
1/32 weight-scale compensation, and adds the bias.

Design (vs the 174.3us baseline; this version: 166.0us cost-model):
- exp(S) is split across TWO engines: ACT runs native Exp->bf16 strips;
  DVE runs a Schraudolph bit-trick exp (int16 = round(a*S + b) written
  via a bitcast AP, whose bits ARE bf16(exp(l))), one tensor_scalar per
  strip. This breaks the baseline's single-engine ACT bottleneck
  (~133us) down to ~96us/engine, making the PE the binding engine.
- All matmuls are plain bf16. Empirically blocked alternatives:
  fp8 anywhere in the q/k path flips softmax argmax in peaky rows
  (logit tails reach 8.5 sigma) and fails the 2e-2 gate; DoubleRow in
  multi-matmul accumulation chains or with explicit tile_position
  crashes the device; GPSIMD cannot access PSUM (walrus verifier).
- Software pipeline: 128 strip units (16 windows x 8 units of 2
  j-chunks); S psum ring depth 3; projections are emitted as <=2-matmul
  "fill pieces" popped between units (alternating mm/pv psum pools
  early) so PE never starves the exp ring; PV runs ~4 windows behind
  with norm-completion-guarded out-projection; the epilogue drains
  out-proj through the then-idle strip psum pool; PE is kept warm
  through the prologue DMA wait by scratch matmuls (the cost model
  halves matmul speed until ~3us of continuous PE busy).

Layouts (per core):
  xT   [128p, 8dc, 2048n] bf16  d = dc*128 + p
  wT   [128p, 8dc, 768e] bf16   e-cols: q-hb0|q-hb1|k-hb0|k-hb1|v(h-maj)
                                group col 64*par+f -> head 2hb+par, feat f
  qkT  [128, 2hb, 2qk, 2048] bf16  partition 64*(h%2)+f
  vv   [128j, 16nt, 4h, 68] bf16  v8 cols 0:64, ones col 64, zero pad
  strips [128j, 2jc, 512i] bf16   p = exp(l)
  oT   [128dl, 2dh, 512i] bf16    attn-out*8; d-slot (dl,dh): h=2dh+dl//64
  woT  [128dl, 2dh, 1024e] bf16   4*w_out.T rows; psum = 32*out
"""

import os
import sys
from contextlib import ExitStack

import numpy as np

for _p in ("/opt/trn_rl_repo", os.path.expanduser("~/.axon_site/_ro/trn_rl_repo")):
    if os.path.isdir(_p) and _p not in sys.path:
        sys.path.append(_p)

import concourse.bass as bass  # noqa: E402
import concourse.mybir as mybir  # noqa: E402
import concourse.tile as tile  # noqa: E402

F32 = mybir.dt.float32
BF16 = mybir.dt.bfloat16
FP8 = mybir.dt.float8e4
U8 = mybir.dt.uint8
EXP = mybir.ActivationFunctionType.Exp
CPY = mybir.ActivationFunctionType.Copy
DR = mybir.MatmulPerfMode.DoubleRow
MULT = mybir.AluOpType.mult
ADD = mybir.AluOpType.add

B, N, DIM, HEADS = 2, 2048, 1024, 16
DH = DIM // HEADS          # 64
NHL = 4                    # heads per core
NCORES = 8
NT = N // 128              # 16 j-chunks
VW = 68                    # v(64) + ones + 3 zero pad (16B-aligned nt stride)
WSCALE = 8.0               # host-side qkv weight scale
LOG2E = 1.4426950408889634
C_SCH = 0.0573             # Schraudolph mantissa bias
A16 = 128.0 * LOG2E / 512.0
B16 = 128.0 * (127.0 - C_SCH)
# The exp bias is encoded INTO the S matmul: the DR pair slots hold
# constants (k: 11.5, q: 1.0) adding exactly 64*11.5 = 736 to every raw
# logit, i.e. p = exp(l - 3.375). DVE then needs only max(A*S', 0)
# (clamped underflow, no uint8 wrap) and ACT a single fixed bias.


# exp engine per strip unit within a window (8 units of 2 jc):
# A=ACT native exp, D=DVE Schraudolph
STRIP_ENG = "AADADADA"       # A5 D3
STRIP_ENG2 = "ADADADAD"      # A4 D4
A4_WINDOWS = {5, 11}


def build_nc(repeat=1, split_waits=True):
    nc = bass.Bass("TRN2", target_bir_lowering=False, debug=False,
                   num_devices=NCORES)
    xT_d = nc.dram_tensor("xT8", [DIM, N], BF16, kind="ExternalInput").ap()
    wT_d = nc.dram_tensor("wT8", [DIM, 768], BF16, kind="ExternalInput").ap()
    woT_d = nc.dram_tensor("woT8", [128, 2, DIM], BF16, kind="ExternalInput").ap()
    id_d = nc.dram_tensor("ident", [128, 128], BF16, kind="ExternalInput").ap()
    out_d = nc.dram_tensor("out", [N, DIM], F32, kind="ExternalOutput").ap()

    with tile.TileContext(nc) as tc, ExitStack() as ctx:
        if repeat > 1:
            ctx.enter_context(tc.For_i(0, repeat, 1))
        pers = ctx.enter_context(tc.tile_pool(name="pers", bufs=1))
        xT = pers.tile([128, 8, N], BF16, tag="xT", name="xT_sb")
        wT = pers.tile([128, 8, 768], BF16, tag="wT", name="wT_sb")
        woT = pers.tile([128, 2, DIM], BF16, tag="woT", name="woT_sb")
        qkT = pers.tile([128, 2, 2, N], BF16, tag="qkT", name="qkT_sb")
        vv = pers.tile([128, NT, NHL, VW], BF16, tag="vv", name="vv_sb")
        ident = pers.tile([128, 128], BF16, tag="id", name="id_sb")

        strip_p = ctx.enter_context(tc.tile_pool(name="strip", bufs=44))
        oT_p = ctx.enter_context(tc.tile_pool(name="oT", bufs=3))
        ostg_p = ctx.enter_context(tc.tile_pool(name="ostg", bufs=6))
        rec_p = ctx.enter_context(tc.tile_pool(name="rec", bufs=2))
        norm_p = ctx.enter_context(tc.tile_pool(name="norm", bufs=2))

        # PSUM: st 3x4KB (6 banks) + mm 1x2KB + pv 1x2KB = 8 banks.
        # st bufs=3 is the exp-pipeline depth (S(u) hazards on exp(u-3));
        # the pv pool also serves the tp transpose tile (alternating).
        st_ps = ctx.enter_context(tc.tile_pool(name="st_ps", bufs=3, space="PSUM"))
        mm_ps = ctx.enter_context(tc.tile_pool(name="mm_ps", bufs=1, space="PSUM"))
        pv_ps = ctx.enter_context(tc.tile_pool(name="pv_ps", bufs=1, space="PSUM"))

        # --- loads: k/q weight cols + x quarter 0 first so the first
        # strips issue early; later x quarters land before their consumers.
        nc.vector.memset(vv[:, :, :, DH:DH + 1], 1.0)
        nc.vector.memset(vv[:, :, :, DH + 1:VW], 0.0)
        xr = xT_d.rearrange("(c p) n -> p c n", p=128)
        wr = wT_d.rearrange("(c p) e -> p c e", p=128)
        nc.sync.dma_start(out=ident[:], in_=id_d)
        nc.sync.dma_start(out=wT[:, :, 256:512], in_=wr[:, :, 256:512])
        for dcp in range(4):
            nc.sync.dma_start(out=xT[:, 2 * dcp:2 * dcp + 2, 0:512],
                              in_=xr[:, 2 * dcp:2 * dcp + 2, 0:512])
        nc.sync.dma_start(out=wT[:, :, 0:256], in_=wr[:, :, 0:256])
        nc.sync.dma_start(out=wT[:, :, 512:768], in_=wr[:, :, 512:768])
        nc.sync.dma_start(out=xT[:, :, 512:1024], in_=xr[:, :, 512:1024])
        nc.sync.dma_start(out=xT[:, :, 1024:1536], in_=xr[:, :, 1024:1536])
        nc.sync.dma_start(out=xT[:, :, 1536:2048], in_=xr[:, :, 1536:2048])
        nc.sync.dma_start(out=woT[:, :, :], in_=woT_d)

        # engine-tagged copy/scale helper: A=ACT, D=DVE
        def eng_copy(eng, out, in_, scale=None):
            if eng == "A":
                nc.scalar.activation(out, in_, CPY,
                                     scale=1.0 if scale is None else scale)
            elif scale is None:
                nc.vector.tensor_copy(out, in_)
            else:
                nc.vector.tensor_scalar(out, in_, scale, None, MULT)

        # --- qkv projections (DoubleRow fp8), built as PIECE LISTS so
        # fills never starve the strip/exp ring; pieces of one group pop
        # consecutively (the mm psum pool has bufs=1).
        # group g4: 0=q-hb0, 1=q-hb1, 2=k-hb0, 3=k-hb1; psum partition
        # 64*par+f holds head 2hb+par feature f, matching qkT's layout.
        def qk_pieces(g4, nb, cp_eng, pool=None, tag="mm"):
            box = [None]
            pool = pool or mm_ps

            def mk(dcp):
                def _p():
                    if dcp == 0:
                        box[0] = pool.tile([128, 512], F32, tag=tag,
                                           name="qk_ps")
                    for u in range(2):
                        dc = 2 * dcp + u
                        nc.tensor.matmul(
                            box[0][:],
                            wT[:, dc, g4 * 128:(g4 + 1) * 128],
                            xT[:, dc, nb * 512:(nb + 1) * 512],
                            start=(dc == 0), stop=(dc == 7))
                return _p

            def cp():
                eng_copy(cp_eng, qkT[:, g4 % 2, g4 // 2,
                                     nb * 512:(nb + 1) * 512], box[0][:])
            return [mk(d) for d in range(4)] + [cp]

        def qk_group(g4, nb, cp_eng):
            for p in qk_pieces(g4, nb, cp_eng):
                p()

        def v_pieces(ntp, cp_eng, pool=None, tag="mm"):
            # two nt chunks into one psum tile -> one copy
            box = [None]
            pool = pool or mm_ps

            def mk(q2, dc2):
                def _p():
                    if q2 == 0 and dc2 == 0:
                        box[0] = pool.tile([128, 2, NHL, DH], F32,
                                           tag=tag, name="v_ps")
                    nt = 2 * ntp + q2
                    for dcp in (2 * dc2, 2 * dc2 + 1):
                        for u in range(2):
                            dc = 2 * dcp + u
                            nc.tensor.matmul(
                                box[0][:, q2, :, :],
                                xT[:, dc, nt * 128:(nt + 1) * 128],
                                wT[:, dc, 512:768],
                                start=(dc == 0), stop=(dc == 7))
                return _p

            def cp():
                eng_copy(cp_eng, vv[:, 2 * ntp:2 * ntp + 2, :, 0:DH],
                         box[0][:])
            return [mk(q2, d) for q2 in range(2) for d in range(2)] + [cp]

        # --- out-projection (bf16) for one (ng, eh) eighth of an i-block
        def oproj_one(oT_prev, it_prev, ng, eh, cp_eng, pool=None, tag="mm"):
            pool = pool or mm_ps
            ps = pool.tile([128, 512], F32, tag=tag, name="op_ps")
            for dh in range(2):
                nc.tensor.matmul(
                    ps[:],
                    oT_prev[:, dh, ng * 128:(ng + 1) * 128],
                    woT[:, dh, eh * 512:(eh + 1) * 512],
                    start=(dh == 0), stop=(dh == 1))
            stg = ostg_p.tile([128, 512], F32, tag="ostg")
            eng_copy(cp_eng, stg[:], ps[:])
            nc.sync.dma_start(
                out=out_d[it_prev * 512 + ng * 128: it_prev * 512 + (ng + 1) * 128,
                          eh * 512:(eh + 1) * 512],
                in_=stg[:])

        # --- prologue: PE warmup during the x DMA (the cost model runs
        # matmuls at half speed until ~3us of continuous PE busy), then
        # k-hb0 nb0 + q-hb0 it0 (window-0/1 strips need only hb0).
        qk_group(2, 0, "A")
        qk_group(0, 0, "A")
        qk_group(2, 1, "A")

        # --- fill schedule: (deadline_unit, [pieces]); <=2 piece pops per
        # unit, popped BEFORE the unit's S matmuls so copies are emitted
        # (= ordered) ahead of their consumers.
        # Unit u = 8*(4it+h)+sg; S(u) reads k(hb=h//2, nb=sg//2) and
        # q(hb, it); PV pops start at u>=38 and need vv complete.
        fillq = []
        # early fill groups alternate mm/pv psum pools (pv is idle until
        # the first PV pop at u>=40) so consecutive groups never stall on
        # one pool's copy turnaround
        fillq.append((-8, qk_pieces(2, 2, "A", pv_ps, "pv")))
        fillq.append((-6, qk_pieces(2, 3, "A")))
        fillq.append((0, qk_pieces(1, 0, "A", pv_ps, "pv")))
        fillq.append((4, qk_pieces(3, 0, "A")))
        fillq.append((6, qk_pieces(3, 1, "A", pv_ps, "pv")))
        fillq.append((8, qk_pieces(3, 2, "A")))
        fillq.append((10, qk_pieces(3, 3, "A", pv_ps, "pv")))
        for ntp in range(8):
            pl = (pv_ps, "pv") if ntp % 2 == 0 and ntp < 6 else (None, "mm")
            fillq.append((13 + 3 * ntp, v_pieces(ntp, "D", pl[0], pl[1])))
        for it in range(1, 4):
            fillq.append((32 * it - 10, qk_pieces(0, it, "A")))
            fillq.append((32 * it + 4, qk_pieces(1, it, "A")))
        fillq.sort(key=lambda f: f[0])
        fillq = [[dl, pieces, len(pieces)] for dl, pieces in fillq]

        pending = []  # out-proj units with deadlines (atomic, use mm pool)

        def pop_fills(u):
            popped = 0
            budget = 3 if (fillq and fillq[0][0] <= u - 6) else 2
            while fillq and fillq[0][0] <= u and popped < budget:
                _, pieces, _n = fillq[0]
                pieces.pop(0)()
                popped += 1
                if not pieces:
                    fillq.pop(0)
            # oproj only when no fill group is mid-flight (mm pool bufs=1)
            # and only after all 4 head norms for its i-block have been
            # emitted (reads oT)
            in_flight = (fillq and len(fillq[0][1]) < fillq[0][2])
            while (pending and pending[0][0] <= u and not in_flight
                   and norm_done[pending[0][2]] >= 4):
                pending.pop(0)[1]()

        norm_done = [0, 0, 0, 0]

        # --- PV + normalize + transpose, a few windows behind ---
        def make_norm(pv_t, oT_t, h, it):
            def _norm():
                rec = rec_p.tile([128, NHL, 1], F32, tag="rec")
                nc.vector.reciprocal(rec[:], pv_t[0][:, :, DH:DH + 1])
                nrm = norm_p.tile([128, NHL, DH], BF16, tag="nrm",
                                  name="nrm_sb")
                in1, in2 = bass.broadcast_tensor_aps(pv_t[0][:, :, 0:DH],
                                                     rec[:])
                nc.vector.tensor_mul(nrm[:], in1, in2)
                tp = pv_ps.tile([128, 512], BF16, tag="pv", name="tp_ps")
                for isub in range(4):
                    nc.tensor.transpose(
                        tp[0:64, isub * 128:(isub + 1) * 128],
                        nrm[:, isub, :], ident[:])
                nc.vector.tensor_copy(
                    oT_t[64 * (h % 2):64 * (h % 2) + 64, h // 2, :],
                    tp[0:64, :])
                norm_done[it] += 1
            return _norm

        def make_pv(strips, h, pv_t, isub, last=None):
            def _pvi():
                if isub == 0:
                    pv_t[0] = pv_ps.tile([128, NHL, 128], F32, tag="pv",
                                         name="pv_ps_t")
                for sgp in range(8):
                    for q2 in range(2):
                        nc.tensor.matmul(
                            pv_t[0][:, isub, 0:DH + 1],
                            strips[sgp][:, q2,
                                        isub * 128:(isub + 1) * 128],
                            vv[:, 2 * sgp + q2, h, 0:DH + 1],
                            start=(sgp == 0 and q2 == 0),
                            stop=(sgp == 7 and q2 == 1))
                if last is not None:
                    last()
            return _pvi

        pvq = []
        strips_w = []
        oT_t = None
        oT_by_it = {}
        stg_rot = "DDDADDDA"
        for u in range(128):
            it, r = divmod(u, 32)
            h, sg = divmod(r, 8)
            w = 4 * it + h
            hb = h // 2
            po = 64 * (h % 2)
            if r == 0:
                oT_t = oT_p.tile([128, 2, 512], BF16, tag="oT", name="oT_t")
                oT_by_it[it] = oT_t
            pop_fills(u)
            ps = st_ps.tile([128, 2, 512], F32, tag="st", name="s_ps")
            for q2 in range(2):
                jc = sg * 2 + q2
                nc.tensor.matmul(
                    ps[:, q2, :],
                    qkT[po:po + 64, hb, 1, jc * 128:(jc + 1) * 128],
                    qkT[po:po + 64, hb, 0, it * 512:(it + 1) * 512],
                    start=True, stop=True)
            strip = strip_p.tile([128, 2, 512], BF16, tag="strip")
            strips_w.append(strip)
            eng = (STRIP_ENG2 if w in A4_WINDOWS else STRIP_ENG)[sg]
            if eng == "A":
                nc.scalar.activation(strip[:], ps[:], EXP,
                                     scale=1.0 / 512.0)
            else:
                nc.vector.tensor_scalar(strip[:].bitcast(mybir.dt.int16),
                                        ps[:], A16, B16, MULT, ADD)
            if u >= 42 and pvq:
                if sg % 2 == 1 or len(pvq) > 4 or u >= 96:
                    pvq.pop(0)()
                if len(pvq) > 6 or (u >= 96 and len(pvq) > 2):
                    pvq.pop(0)()

            if sg == 7:
                pv_t = [None]
                norm = make_norm(pv_t, oT_t, h, it)
                for isub in range(4):
                    pvq.append(make_pv(list(strips_w), h, pv_t, isub,
                                       norm if isub == 3 else None))
                strips_w.clear()
                if h == 3:
                    # oproj(it) gated by norm(it,h3); oT bufs=3 so the
                    # hard deadline is oT(it+3)'s alloc at unit 32it+96
                    for k in range(8):
                        pending.append((32 * it + 64 + k,
                            lambda oT_prev=oT_t, it_prev=it, k=k:
                            oproj_one(oT_prev, it_prev, k // 2, k % 2,
                                      stg_rot[k]), it))
        while pvq:
            pvq.pop(0)()
        # epilogue: the st ring is idle; run the remaining out-proj units
        # through it (3 bufs) so mm-pool rotation stops serializing them
        for i, (dl, fn, it_prev) in enumerate(pending):
            k = dl - 64 - 32 * it_prev
            oproj_one(oT_by_it[it_prev], it_prev, k // 2, k % 2,
                      stg_rot[k], pool=st_ps, tag="st")
        pending.clear()
    if split_waits:
        _split_dma_waits(nc)
    return nc


def _split_dma_waits(nc):
    """walrus's DMA/LDWEIGHTS encodings take a single sync wait; move
    extra waits onto an EventSemaphore on the issuing sequencer."""
    fn = nc.m.functions[0]
    for bb in fn.blocks:
        insts = bb.instructions
        i = 0
        while i < len(insts):
            inst = insts[i]
            si = getattr(inst, "sync_info", None)
            if (si is not None and len(si.on_wait) > 1
                    and type(inst).__name__ != "InstEventSemaphore"):
                waits = list(si.on_wait)
                for k, w in enumerate(waits[:-1]):
                    ev = mybir.InstEventSemaphore(
                        name=f"{inst.name}-wsplit{k}", ins=[], outs=[])
                    ev.engine = inst.engine
                    ev.sync_info = type(si)(on_wait=[w], on_update=[])
                    insts.insert(i, ev)
                    i += 1
                inst.sync_info = type(si)(on_wait=waits[-1:],
                                          on_update=list(si.on_update))
            i += 1


_NC = None


def _get_nc():
    global _NC
    if _NC is None:
        _NC = build_nc()
    return _NC


def make_in_maps(x, w_qkv, w_out):
    import ml_dtypes as _md
    f8 = _md.float8_e4m3
    x = np.asarray(x, dtype=np.float32)
    w_qkv = np.asarray(w_qkv, dtype=np.float32)
    w_out = np.asarray(w_out, dtype=np.float32)
    xT_by_b = [np.ascontiguousarray(x[b].T).astype(_md.bfloat16) for b in range(B)]
    ident = np.eye(128, dtype=_md.bfloat16)
    in_maps = []
    for c in range(NCORES):
        b, g = divmod(c, 4)
        r0 = g * NHL * DH  # 256-row slice of each of q/k/v sections
        # group g4=(qk,hb): col 64*par + f -> head 2hb+par, feature f
        rows = []
        for g4 in range(4):
            qk, hb = g4 // 2, g4 % 2
            perm = np.empty(128, np.int32)
            for par in range(2):
                for f in range(DH):
                    perm[64 * par + f] = (2 * hb + par) * DH + f
            rows.append(w_qkv[qk * DIM + r0 + perm])
        wv = w_qkv[2 * DIM + r0:2 * DIM + r0 + 256]   # h-major natural
        wT = np.ascontiguousarray(
            (WSCALE * np.concatenate(rows + [wv], 0)).T).astype(_md.bfloat16)
        # woT[dl, dh, e] = w_out[e, r0 + (2dh + dl//64)*64 + dl%64]
        woT = np.empty((128, 2, DIM), np.float32)
        for dh in range(2):
            for hl in range(2):
                hh = 2 * dh + hl
                woT[64 * hl:64 * hl + 64, dh, :] = (
                    4.0 * w_out[:, r0 + hh * 64:r0 + hh * 64 + 64].T)
        in_maps.append({"xT8": xT_by_b[b], "wT8": wT,
                        "woT8": woT.astype(_md.bfloat16), "ident": ident})
    return in_maps


def combine(results, b_out):
    """results: list of 8 dicts with 'out' [N, DIM] fp32 partials."""
    b_out = np.asarray(b_out, dtype=np.float32)
    out = np.empty((B, N, DIM), dtype=np.float32)
    for b in range(B):
        acc = results[4 * b]["out"].astype(np.float32)
        for g in range(1, 4):
            acc += results[4 * b + g]["out"].astype(np.float32)
        out[b] = acc * (1.0 / 32.0) + b_out[None, :]
    return out


def kernel(x, w_qkv, w_out, b_out):
    from concourse.bass_utils import run_bass_kernel_spmd
    nc = _get_nc()
    in_maps = make_in_maps(x, w_qkv, w_out)
    res = run_bass_kernel_spmd(nc, in_maps, list(range(NCORES)))
    return combine(res.results, b_out)
